# revision 44
# baseline (speedup 1.0000x reference)
"""Trainium2 Bass kernel for nn_AggrHGraphConvWindow (3x GraphConv -> LeakyReLU -> 2-layer LSTM).

Contract: kernel(**inputs) takes FULL unsharded numpy inputs, returns FULL output
(33500, 16, 128) float32.  Internally shards destination rows across 8 NeuronCores
(graph/data parallel per the sharding hint: edges partitioned by destination with
halo exchange of source features), runs one SPMD Bass program, and gathers.
"""

import os
import numpy as np
import ml_dtypes

BF16 = np.float16  # fp16: same cost as bf16 on PE/DVE, 8x finer mantissa
FP8 = ml_dtypes.float8_e4m3

# Problem constants (hardcoded per spec)
N_NODE, N_POD, N_SVC = 500, 30000, 3000
T, F, H = 16, 64, 128
NCORES = 8
P = 128

NODE_PC = 64     # nodes per core (64*8=512 >= 500)
POD_PC = 3750    # pods per core (exact)
SVC_PC = 376     # svcs per core (376*8=3008 >= 3000)

NODE_TILES = 1   # 64 real rows inside one 128-row tile
POD_TILES = (POD_PC + P - 1) // P   # 30
SVC_TILES = (SVC_PC + P - 1) // P   # 3
N_TILES = NODE_TILES + POD_TILES + SVC_TILES  # 34
R_CORE = N_TILES * P  # 4352 rows per core (padded)

# LSTM batch tiles over the 4352 local rows
LSTM_TILES = [(j * 512, 512) for j in range(R_CORE // 512)]
if R_CORE % 512:
    LSTM_TILES.append((512 * (R_CORE // 512), R_CORE % 512))
NJ = len(LSTM_TILES)
QUADS = [(0, 1, 2, 3), (4, 5, 6, 7), (8,)]

_COMPILED = {}


# ----------------------------------------------------------------------------
# Host-side preprocessing: edge routing, degree norms, halo tables, weight prep
# ----------------------------------------------------------------------------

def _degrees(src, dst, n_src, n_dst):
    dout = np.bincount(src, minlength=n_src).astype(np.float64)
    din = np.bincount(dst, minlength=n_dst).astype(np.float64)
    return (1.0 / np.sqrt(np.maximum(dout, 1.0)), 1.0 / np.sqrt(np.maximum(din, 1.0)))


def _prep(inputs):
    nf = np.asarray(inputs["node_feat"]).reshape(N_NODE, T * F)
    pf = np.asarray(inputs["pod_feat"]).reshape(N_POD, T * F)
    sf = np.asarray(inputs["svc_feat"]).reshape(N_SVC, T * F)

    in_src = np.asarray(inputs["inst_node_src"]).astype(np.int64)
    in_dst = np.asarray(inputs["inst_node_dst"]).astype(np.int64)
    ni_src = np.asarray(inputs["node_inst_src"]).astype(np.int64)
    ni_dst = np.asarray(inputs["node_inst_dst"]).astype(np.int64)
    sc_src = np.asarray(inputs["svc_call_src"]).astype(np.int64)
    sc_dst = np.asarray(inputs["svc_call_dst"]).astype(np.int64)

    # normalization: x/sqrt(deg_out) -> segsum -> /sqrt(deg_in), folded per-edge
    ro_in, ri_in = _degrees(in_src, in_dst, N_POD, N_NODE)
    ro_ni, ri_ni = _degrees(ni_src, ni_dst, N_NODE, N_POD)
    ro_sc, ri_sc = _degrees(sc_src, sc_dst, N_SVC, N_SVC)

    # Route edges: per (core, tile) buckets.
    # tile order within core: pods tiles 0..29, svc 30..32, node 33 (node last)
    def route(src, dst, w, kind):
        if kind == 0:    # dst = node -> last tile (heaviest; keeps LSTM ramp fast)
            core = dst // NODE_PC
            q = dst - core * NODE_PC
            tile = np.full_like(dst, POD_TILES + SVC_TILES)
            row = q
        elif kind == 1:  # dst = pod -> tiles [0, POD_TILES)
            core = dst // POD_PC
            q = dst - core * POD_PC
            tile = q // P
            row = q % P
        else:            # dst = svc -> tiles [POD_TILES, POD_TILES+SVC_TILES)
            core = dst // SVC_PC
            q = dst - core * SVC_PC
            tile = POD_TILES + q // P
            row = q % P
        return core, tile, row, src, w

    ew_in = (ro_in[in_src] * ri_in[in_dst]).astype(np.float32)
    ew_ni = (ro_ni[ni_src] * ri_ni[ni_dst]).astype(np.float32)
    ew_sc = (ro_sc[sc_src] * ri_sc[sc_dst]).astype(np.float32)

    routed = {
        0: route(in_src, in_dst, ew_in, 0),   # node phase: src = pods
        1: route(ni_src, ni_dst, ew_ni, 1),   # pod phase:  src = nodes
        2: route(sc_src, sc_dst, ew_sc, 2),   # svc phase:  src = svcs
    }

    # per (core, tile) edge lists
    buckets = [[([], [], []) for _ in range(N_TILES)] for _ in range(NCORES)]
    for kind in (0, 1, 2):
        core, tile, row, src, w = routed[kind]
        order = np.lexsort((row, tile, core))
        core, tile, row, src, w = core[order], tile[order], row[order], src[order], w[order]
        # group
        key = core * N_TILES + tile
        uniq, starts = np.unique(key, return_index=True)
        starts = list(starts) + [len(key)]
        for ui, k in enumerate(uniq):
            c, t = int(k) // N_TILES, int(k) % N_TILES
            s, e = starts[ui], starts[ui + 1]
            buckets[c][t] = (src[s:e], row[s:e], w[s:e])

    # static chunk counts per tile (max over cores), >= 1
    K = []
    for t in range(N_TILES):
        mx = 1
        for c in range(NCORES):
            mx = max(mx, (len(buckets[c][t][0]) + P - 1) // P)
        K.append(mx)
    base = np.concatenate([[0], np.cumsum(K)]).astype(np.int64)
    C_total = int(base[-1])

    # Source features laid out in EDGE ORDER (row-duplicated): chunk c of the
    # conv reads srcdup[c*128:(c+1)*128] with a plain contiguous DMA -- no
    # indirect gather (saves the SWDGE descriptor-generation cost on gpsimd).
    srcfeat = {0: pf, 1: nf, 2: sf}
    kind_num = [1] * POD_TILES + [2] * SVC_TILES + [0] * NODE_TILES

    in_maps = []
    for c in range(NCORES):
        edst = np.zeros((C_total, P), dtype=np.float32)
        ew = np.zeros((C_total, P), dtype=np.float32)
        srcdup = np.zeros((C_total * P, T * F), dtype=BF16)

        for t in range(N_TILES):
            src, row, w = buckets[c][t]
            n = len(src)
            b0 = int(base[t]) * P
            if n:
                srcdup[b0:b0 + n] = srcfeat[kind_num[t]][src].astype(BF16)
            edst.reshape(-1)[b0:b0 + n] = row
            ew.reshape(-1)[b0:b0 + n] = w

        m = {
            "srcdup": srcdup,
            "edst": np.ascontiguousarray(edst.T),
            "ew": np.ascontiguousarray(ew.T),
        }
        in_maps.append(m)

    # ---- weights (identical on all cores) ----
    def conv_w(Wname):
        W = np.asarray(inputs[Wname])  # (T, F, H)
        wt = W.transpose(1, 0, 2).reshape(F, T * H)  # (64, 2048) F-major
        return np.vstack([wt, wt]).astype(BF16)       # (128, 2048) vertical dup

    def conv_b8(bname):
        # per-t DoubleRow rank-1 bias. Both DR slots are used for a
        # two-term compensated sum: hi = fp8(b), lo = fp8(b - hi); the
        # matmul adds them, cutting fp8 quantization error ~16x for free.
        b = np.asarray(inputs[bname]).reshape(T, H)
        hi = b.astype(FP8)
        lo = (b - hi.astype(np.float32)).astype(FP8)
        out = np.empty((T, 2, H), dtype=FP8)
        out[:, 0, :] = hi
        out[:, 1, :] = lo
        return out.reshape(1, T * 2 * H)

    def lstm_w(Wname):
        # rows [i,f,g,o] -> [i,f,o,g]; g block doubled so tanh(g) = 2*sigmoid(2g)-1
        # lets one Sigmoid cover all four gate chunks.
        W = np.asarray(inputs[Wname])  # (512, in_dim)
        Wp = np.concatenate([W[0:128], W[128:256], W[384:512], 2.0 * W[256:384]], axis=0)
        return np.ascontiguousarray(Wp.T).astype(BF16)  # (in_dim, 512), [i,f,o,2g]

    def lstm_b8(b1, b2):
        b = np.asarray(inputs[b1]) + np.asarray(inputs[b2])
        bp = np.concatenate([b[0:128], b[128:256], b[384:512], 2.0 * b[256:384]])
        bp = bp.reshape(4, H)
        hi = bp.astype(FP8)
        lo = (bp - hi.astype(np.float32)).astype(FP8)
        out = np.empty((4, 2, H), dtype=FP8)
        out[:, 0, :] = hi
        out[:, 1, :] = lo
        return out.reshape(1, 4 * 2 * H)

    shared = {
        "wt_node": conv_w("W_in"), "wt_pod": conv_w("W_ni"), "wt_svc": conv_w("W_svc"),
        "cb8_node": conv_b8("b_in"), "cb8_pod": conv_b8("b_ni"), "cb8_svc": conv_b8("b_svc"),
        "wih0": lstm_w("Wih0"), "whh0": lstm_w("Whh0"),
        "wih1": lstm_w("Wih1"), "whh1": lstm_w("Whh1"),
        "bias8_0": lstm_b8("bih0", "bhh0"), "bias8_1": lstm_b8("bih1", "bhh1"),
        "ones8": np.ones((1, 1024), dtype=FP8),
        "iota": np.broadcast_to(np.arange(P, dtype=np.float32), (P, P)).copy(),
    }
    for m in in_maps:
        m.update(shared)

    meta = (C_total, tuple(K))
    return meta, in_maps


# ----------------------------------------------------------------------------
# Device program
# ----------------------------------------------------------------------------

def _build(meta):
    import concourse.bass as bass
    import concourse.tile as tile
    import concourse.mybir as mybir

    C_total, K = meta
    f32 = mybir.dt.float32
    bf16 = mybir.dt.float16
    fp16 = mybir.dt.float16
    fp8 = mybir.dt.float8e4
    i32 = mybir.dt.int32
    AF = mybir.ActivationFunctionType
    ALU = mybir.AluOpType
    DR = mybir.MatmulPerfMode.DoubleRow

    import concourse.bacc as bacc
    nc = bacc.Bacc("TRN2", target_bir_lowering=False, debug=False, enable_asserts=False)

    srcdup_d = nc.dram_tensor("srcdup", [C_total * P, T * F], bf16, kind="ExternalInput")
    edst_d = nc.dram_tensor("edst", [P, C_total], f32, kind="ExternalInput")
    ew_d = nc.dram_tensor("ew", [P, C_total], f32, kind="ExternalInput")
    wt_d = {k: nc.dram_tensor(f"wt_{k}", [P, T * H], bf16, kind="ExternalInput")
            for k in ("node", "pod", "svc")}
    cb8_d = {k: nc.dram_tensor(f"cb8_{k}", [1, T * 2 * H], fp8, kind="ExternalInput")
             for k in ("node", "pod", "svc")}
    wih_d = [nc.dram_tensor(f"wih{l}", [H, 512], bf16, kind="ExternalInput") for l in range(2)]
    whh_d = [nc.dram_tensor(f"whh{l}", [H, 512], bf16, kind="ExternalInput") for l in range(2)]
    bias8_d = [nc.dram_tensor(f"bias8_{l}", [1, 4 * 2 * H], fp8, kind="ExternalInput") for l in range(2)]
    ones8_d = nc.dram_tensor("ones8", [1, 1024], fp8, kind="ExternalInput")
    iota_d = nc.dram_tensor("iota", [P, P], f32, kind="ExternalInput")
    out_d = nc.dram_tensor("out", [P, T * R_CORE], bf16, kind="ExternalOutput")

    tile_kind = (["pod"] * POD_TILES + ["svc"] * SVC_TILES + ["node"] * NODE_TILES)
    base = np.concatenate([[0], np.cumsum(K)]).astype(int)
    NODE_TILE_IDX = POD_TILES + SVC_TILES

    with tile.TileContext(nc) as tc:
        with tc.tile_pool(name="dram", bufs=NJ, space="DRAM") as dramp, \
             tc.tile_pool(name="const", bufs=1) as constp:
            # x0 spill split per LSTM batch tile so the LSTM can start on tile j
            # as soon as its 4 conv row-tiles are written (pipeline the phases)
            x0p = [dramp.tile([P, T * B], bf16, tag="x0p", name=f"x0p_{j}")
                   for j, (r0, B) in enumerate(LSTM_TILES)]

            # load constants
            edst_sb = constp.tile([P, C_total], f32)
            ew_sb = constp.tile([P, C_total], f32)
            iota_sb = constp.tile([P, P], f32)
            nc.sync.dma_start(edst_sb[:], edst_d.ap())
            nc.sync.dma_start(ew_sb[:], ew_d.ap())
            nc.sync.dma_start(iota_sb[:], iota_d.ap())
            wt_sb, cb8_sb = {}, {}
            for k in ("node", "pod", "svc"):
                wt_sb[k] = constp.tile([P, T * H], bf16, name=f"wt_{k}_sb")
                cb8_sb[k] = constp.tile([1, T * 2 * H], fp8, name=f"cb8_{k}_sb")
                nc.sync.dma_start(wt_sb[k][:], wt_d[k].ap())
                nc.sync.dma_start(cb8_sb[k][:], cb8_d[k].ap())
            wih_sb, whh_sb, bias8_sb = [], [], []
            for l in range(2):
                wih_sb.append(constp.tile([H, 512], bf16, name=f"wih{l}_sb"))
                whh_sb.append(constp.tile([H, 512], bf16, name=f"whh{l}_sb"))
                bias8_sb.append(constp.tile([1, 4 * 2 * H], fp8, name=f"bias8_{l}_sb"))
                nc.sync.dma_start(wih_sb[l][:], wih_d[l].ap())
                nc.sync.dma_start(whh_sb[l][:], whh_d[l].ap())
                nc.sync.dma_start(bias8_sb[l][:], bias8_d[l].ap())
            ones8_sb = constp.tile([1, 1024], fp8)
            nc.sync.dma_start(ones8_sb[:], ones8_d.ap())

            srcdup_ap = srcdup_d.ap()

            # Conv + LSTM share one scope (and one PSUM pool) so the two
            # phases pipeline: LSTM batch-tile j starts once its 4 conv
            # row-tiles have spilled.
            with tc.tile_pool(name="gat", bufs=6) as gatp, \
                 tc.tile_pool(name="ssb", bufs=6) as ssbp, \
                 tc.tile_pool(name="psum", bufs=2, space="PSUM") as psump, \
                 tc.tile_pool(name="aggsb", bufs=3) as aggsbp, \
                 tc.tile_pool(name="x0sb", bufs=3) as x0sbp, \
                 tc.tile_pool(name="x0res", bufs=3) as x0resp, \
                 tc.tile_pool(name="st_h0", bufs=NJ) as ph0, \
                 tc.tile_pool(name="st_c", bufs=2) as pcm, \
                 tc.tile_pool(name="st_h1", bufs=NJ) as ph1, \
                 tc.tile_pool(name="ifo", bufs=3) as ifop, \
                 tc.tile_pool(name="gt", bufs=4) as gtp, \
                 tc.tile_pool(name="og", bufs=11) as otp, \
                 tc.tile_pool(name="s2c", bufs=2) as s2cp, \
                 tc.tile_pool(name="tmp1", bufs=3) as t1p, \
                 tc.tile_pool(name="tmp2", bufs=3) as t2p, \
                 tc.tile_pool(name="xin", bufs=6) as xinp:

                def bias_mm(out_ap, lhsT_flat, n, start, stop):
                    # rank-1 bias add at half PE cost: fp8 DoubleRow with
                    # lhsT = [bias;0] pairs, rhs = ones
                    nc.tensor.matmul(
                        out=out_ap,
                        lhsT=lhsT_flat.rearrange("o (two m) -> o two m", two=2),
                        rhs=ones8_sb[:, 0:2 * n].rearrange("o (two m) -> o two m", two=2),
                        start=start, stop=stop, perf_mode=DR)

                def conv_group(d, k0, k1, gs, ss, agg, R):
                    # PE-accumulate chunk group [k0,k1] into one PSUM partial
                    # (m-outer so per-bank psum accumulation groups stay
                    # sequential), then DVE-combine into agg.
                    pp = psump.tile([P, T * H], f32, tag="ps", name=f"pp_{d}_{k0}")
                    for m in range(8):
                        for kk in range(k0, k1 + 1):
                            nc.tensor.matmul(
                                out=pp[:, m * R:(m + 1) * R],
                                lhsT=gs[kk][:, m * P:(m + 1) * P],
                                rhs=ss[kk][:], start=(kk == k0), stop=(kk == k1))
                    if k0 == 0:
                        nc.vector.tensor_copy(agg[:], pp[:, 0:8 * R])
                    else:
                        nc.vector.tensor_tensor(out=agg[:], in0=agg[:],
                                                in1=pp[:, 0:8 * R], op=ALU.add)

                N_RES = 3  # LSTM batch tiles whose x0 stays SBUF-resident
                x0res = [x0resp.tile([P, T * 512], bf16, tag="x0r", name=f"x0res_{j}")
                         for j in range(N_RES)]

                def conv_tail(d, kind, agg, R):
                    # linear per timestep + bias (fp8 DoubleRow rank-1) -> PSUM,
                    # then LeakyReLU(0.01) -> fp16 -> spill (or SBUF-resident)
                    hx = psump.tile([P, T * H], f32, tag="ps", name=f"hx_{d}")
                    wt = wt_sb[kind]
                    for t in range(T):
                        pb = 64 * (t % 2)
                        bias_mm(hx[:, t * R:(t + 1) * R],
                                cb8_sb[kind][:, t * 2 * H:(t + 1) * 2 * H], R,
                                start=True, stop=False)
                        nc.tensor.matmul(
                            out=hx[:, t * R:(t + 1) * R],
                            lhsT=wt[pb:pb + F, t * H:(t + 1) * H],
                            rhs=agg[pb:pb + F, (t // 2) * R:(t // 2 + 1) * R],
                            start=False, stop=True)
                    j = min(d // 4, NJ - 1)
                    rl = P * (d - 4 * j)
                    if j < N_RES:
                        # Prelu writes straight into the resident tile's
                        # [h, t*512 + rl + r] slices; no DRAM round trip
                        dst = x0res[j][:].rearrange("h (t r) -> h t r", t=T)[:, :, rl:rl + R]
                        nc.scalar.activation(
                            dst, hx[:, 0:T * R].rearrange("h (t r) -> h t r", t=T),
                            AF.Prelu, alpha=0.01)
                        return
                    x0t = x0sbp.tile([P, T * R], bf16, tag="x0", name=f"x0t_{d}")
                    nc.scalar.activation(x0t[:], hx[:, 0:T * R], AF.Prelu, alpha=0.01)
                    # spill as one contiguous per-partition block (4KB runs, no
                    # sub-512B DMA penalty in the DMA-bound conv window); the
                    # strided cost moves to the LSTM-phase load where DMA is idle
                    nc.sync.dma_start(
                        x0p[j][:, rl * T:rl * T + R * T], x0t[:])

                gather_rr = [0]

                def conv_tile(d):
                    kind = tile_kind[d]
                    Kd = K[d]
                    R = P
                    agg = aggsbp.tile([P, 8 * R], bf16, tag="agg", name=f"agg_{d}")
                    gs, ss = [], []
                    for ki in range(Kd):
                        col = int(base[d]) + ki
                        g = gatp.tile([P, T * F], bf16, tag="g", name=f"g_{d}_{ki}")
                        # round-robin the gather issue over 2 DGE queues
                        eng = (nc.sync, nc.scalar)[gather_rr[0] % 2]
                        gather_rr[0] += 1
                        eng.dma_start(g[:], srcdup_ap[col * P:(col + 1) * P, :])
                        s = ssbp.tile([P, R], bf16, tag="s", name=f"s_{d}_{ki}")
                        nc.vector.tensor_scalar(
                            out=s[:], in0=iota_sb[:, 0:R],
                            scalar1=edst_sb[:, col:col + 1], scalar2=ew_sb[:, col:col + 1],
                            op0=ALU.is_equal, op1=ALU.mult)
                        gs.append(g)
                        ss.append(s)
                        if ki % 4 == 3 or ki == Kd - 1:
                            conv_group(d, (ki // 4) * 4, ki, gs, ss, agg, R)
                    conv_tail(d, kind, agg, R)

                # ---------------- LSTM phase ----------------
                h = [[None] * NJ, [None] * NJ]
                for j, (r0, B) in enumerate(LSTM_TILES):
                    h[0][j] = ph0.tile([P, B], bf16, tag="h0", name=f"h0_{j}")
                    h[1][j] = ph1.tile([P, B], bf16, tag="h1", name=f"h1_{j}")
                # merged cell-state per layer: [P, (j,B)] fp16, slices at j*512
                cm = [pcm.tile([P, 4608], fp16, tag="c", name=f"c_{l}") for l in range(2)]

                outr = out_d.ap().rearrange("h (t r) -> h t r", t=T)

                def cell_front(l, j, t, xin_l, os_, crit=False):
                    # gates [i,f,o,2g] via PE; one Sigmoid covers all four
                    # chunks (g weights pre-doubled; tanh(g) = 2*sigmoid(2g)-1).
                    # The freshest operand goes LAST in the matmul group: L0's
                    # hh (h from the sigma2c at this block's start), L1's ih
                    # (h0 of the same t) -- PE pre-runs the other matmuls.
                    B = LSTM_TILES[j][1]
                    gates = psump.tile([P, 4 * B], f32, tag="ps",
                                       name=f"gates_{l}_{j}_{t}")
                    for i in range(4):
                        bias_mm(gates[:, i * B:(i + 1) * B],
                                bias8_sb[l][:, i * 2 * H:(i + 1) * 2 * H], B,
                                start=True, stop=False)
                        mms = [(wih_sb[l], xin_l)]
                        if t > 0:
                            hh = (whh_sb[l], h[l][j][:])
                            mms = [hh, mms[0]] if l == 1 else [mms[0], hh]
                        for mi, (w, rhs) in enumerate(mms):
                            nc.tensor.matmul(
                                out=gates[:, i * B:(i + 1) * B],
                                lhsT=w[:, i * H:(i + 1) * H],
                                rhs=rhs, start=False, stop=(mi == len(mms) - 1))
                    sg = ifop.tile([P, 4 * B], bf16, tag="ifo", name=f"sg_{l}_{j}_{t}")
                    nc.scalar.activation(sg[:], gates[:], AF.Sigmoid)
                    # o-gate extracted so sg can recycle fast (small ring)
                    o = otp.tile([P, B], bf16, tag="o", name=f"o_{l}_{j}_{t}")
                    nc.vector.tensor_copy(o[:], sg[:, 2 * B:3 * B])
                    os_[j] = o
                    gt = gtp.tile([P, B], bf16, tag="gt", name=f"gt_{l}_{j}_{t}")
                    nc.vector.tensor_scalar(
                        out=gt[:], in0=sg[:, 3 * B:4 * B], scalar1=2.0, scalar2=-1.0,
                        op0=ALU.mult, op1=ALU.add)
                    cs = cm[l][:, j * 512:j * 512 + B]
                    if t == 0:
                        nc.vector.tensor_mul(cs, sg[:, 0:B], gt[:])
                    else:
                        t1 = t1p.tile([P, B], fp16, tag="t1", name=f"t1_{l}_{j}_{t}")
                        nc.vector.tensor_mul(t1[:], sg[:, B:2 * B], cs)
                        t2 = t2p.tile([P, B], bf16, tag="t2", name=f"t2_{l}_{j}_{t}")
                        nc.vector.tensor_mul(t2[:], sg[:, 0:B], gt[:])
                        if crit:
                            nc.vector.tensor_tensor(cs, t1[:], t2[:], op=ALU.add)
                        else:
                            nc.gpsimd.tensor_tensor(cs, t1[:], t2[:], op=ALU.add)

                def sigma2c_tail(l, t, quad, os_):
                    # merged tanh over the quad's c columns in ONE activation
                    # call, then h = sig(o)*tanh(c) per tile
                    q0 = min(quad) * 512
                    q1 = max(j * 512 + LSTM_TILES[j][1] for j in quad)
                    tcm = s2cp.tile([P, q1 - q0], bf16, tag="s2", name=f"tc_{l}_{t}_{quad[0]}")
                    nc.scalar.activation(tcm[:], cm[l][:, q0:q1], AF.Tanh)
                    for j in quad:
                        r0, B = LSTM_TILES[j]
                        nc.vector.tensor_mul(
                            h[l][j][:], os_[j][:], tcm[:, j * 512 - q0:j * 512 - q0 + B])
                        if l == 1:
                            nc.sync.dma_start(
                                outr[:, t:t + 1, r0:r0 + B].rearrange("h t r -> h (t r)"),
                                h[1][j][:])

                # per-t emission; q1/q2 sigma2c tails deferred into the next
                # layer's Act stream so Act never stalls on the c-update chain
                pending = [None]

                def lstm_t(t):
                    xs = {}
                    for j in range(NJ):
                        if j < N_RES:
                            xs[j] = x0res[j][:, t * 512:(t + 1) * 512]
                            continue
                        B = LSTM_TILES[j][1]
                        x = xinp.tile([P, B], bf16, tag="x", name=f"x_{t}_{j}")
                        nc.sync.dma_start(
                            x[:].rearrange("h (dl t r) -> h dl t r", t=1, r=P),
                            x0p[j][:].rearrange("h (dl t r) -> h dl t r", t=T, r=P)
                            [:, :, t:t + 1, :])
                        xs[j] = x
                    for l in range(2):
                        os_ = {}
                        for j in (0, 1):
                            cell_front(l, j, t, xs[j] if l == 0 else h[0][j][:], os_)
                        if pending[0] is not None:
                            pl, pt, pos = pending[0]
                            sigma2c_tail(pl, pt, QUADS[1], pos)
                            sigma2c_tail(pl, pt, QUADS[2], pos)
                            pending[0] = None
                        cell_front(l, 2, t, xs[2] if l == 0 else h[0][2][:], os_)
                        cell_front(l, 3, t, xs[3] if l == 0 else h[0][3][:], os_,
                                   crit=True)
                        cell_front(l, 8, t, xs[8] if l == 0 else h[0][8][:], os_,
                                   crit=True)
                        sigma2c_tail(l, t, QUADS[0], os_)
                        for j in (4, 5, 6, 7):
                            cell_front(l, j, t, xs[j] if l == 0 else h[0][j][:], os_,
                                       crit=(j >= 6))
                        pending[0] = (l, t, os_)

                for d in range(N_TILES):
                    conv_tile(d)
                for t in range(T):
                    lstm_t(t)
                pl, pt, pos = pending[0]
                sigma2c_tail(pl, pt, QUADS[1], pos)
                sigma2c_tail(pl, pt, QUADS[2], pos)

    nc.compile()
    return nc


# ----------------------------------------------------------------------------
# Entry points
# ----------------------------------------------------------------------------

def _assemble(results):
    # per-core out: (128, T*R_CORE) viewed [h, t*R_CORE + r] -> (r, t, h)
    full = np.empty((N_NODE + N_POD + N_SVC, T, H), dtype=np.float32)
    parts_node, parts_pod, parts_svc = [], [], []
    for cidx, res in enumerate(results):
        o = res["out"].astype(np.float32).reshape(H, T, R_CORE).transpose(2, 1, 0)  # (r, t, h)
        n_node = min(NODE_PC, max(0, N_NODE - cidx * NODE_PC))
        n_svc = min(SVC_PC, max(0, N_SVC - cidx * SVC_PC))
        parts_pod.append(o[0:POD_PC])
        svc0 = POD_TILES * P
        parts_svc.append(o[svc0:svc0 + n_svc])
        node0 = (POD_TILES + SVC_TILES) * P
        parts_node.append(o[node0:node0 + n_node])
    full[0:N_NODE] = np.concatenate(parts_node, axis=0)
    full[N_NODE:N_NODE + N_POD] = np.concatenate(parts_pod, axis=0)
    full[N_NODE + N_POD:] = np.concatenate(parts_svc, axis=0)
    return full


def run(inputs, trace=False):
    from concourse.bass_utils import run_bass_kernel_spmd
    meta, in_maps = _prep(inputs)
    if meta not in _COMPILED:
        _COMPILED[meta] = _build(meta)
    nc = _COMPILED[meta]
    try:
        res = run_bass_kernel_spmd(nc, in_maps, core_ids=list(range(NCORES)), trace=trace)
    except Exception:
        # transient device errors (e.g. NRT_EXEC_UNIT_UNRECOVERABLE) recover
        # on re-execution; retry once before giving up
        res = run_bass_kernel_spmd(nc, in_maps, core_ids=list(range(NCORES)), trace=trace)
    return _assemble(res.results), res


def kernel(**inputs):
    out, _ = run(inputs, trace=False)
    return out


# revision 48
# speedup vs baseline: 1.0037x; 1.0037x over previous
"""Trainium2 Bass kernel for nn_AggrHGraphConvWindow (3x GraphConv -> LeakyReLU -> 2-layer LSTM).

Contract: kernel(**inputs) takes FULL unsharded numpy inputs, returns FULL output
(33500, 16, 128) float32.  Internally shards destination rows across 8 NeuronCores
(graph/data parallel per the sharding hint: edges partitioned by destination with
halo exchange of source features), runs one SPMD Bass program, and gathers.
"""

import os
import numpy as np
import ml_dtypes

BF16 = np.float16  # fp16: same cost as bf16 on PE/DVE, 8x finer mantissa
FP8 = ml_dtypes.float8_e4m3

# Problem constants (hardcoded per spec)
N_NODE, N_POD, N_SVC = 500, 30000, 3000
T, F, H = 16, 64, 128
NCORES = 8
P = 128

NODE_PC = 64     # nodes per core (64*8=512 >= 500)
POD_PC = 3750    # pods per core (exact)
SVC_PC = 376     # svcs per core (376*8=3008 >= 3000)

NODE_TILES = 1   # 64 real rows inside one 128-row tile
POD_TILES = (POD_PC + P - 1) // P   # 30
SVC_TILES = (SVC_PC + P - 1) // P   # 3
N_TILES = NODE_TILES + POD_TILES + SVC_TILES  # 34
R_CORE = N_TILES * P  # 4352 rows per core (padded)

# LSTM batch tiles over the 4352 local rows
LSTM_TILES = [(j * 512, 512) for j in range(R_CORE // 512)]
if R_CORE % 512:
    LSTM_TILES.append((512 * (R_CORE // 512), R_CORE % 512))
NJ = len(LSTM_TILES)
QUADS = [(0, 1, 2, 3), (4, 5, 6, 7), (8,)]

_COMPILED = {}


# ----------------------------------------------------------------------------
# Host-side preprocessing: edge routing, degree norms, halo tables, weight prep
# ----------------------------------------------------------------------------

def _degrees(src, dst, n_src, n_dst):
    dout = np.bincount(src, minlength=n_src).astype(np.float64)
    din = np.bincount(dst, minlength=n_dst).astype(np.float64)
    return (1.0 / np.sqrt(np.maximum(dout, 1.0)), 1.0 / np.sqrt(np.maximum(din, 1.0)))


def _prep(inputs):
    nf = np.asarray(inputs["node_feat"]).reshape(N_NODE, T * F)
    pf = np.asarray(inputs["pod_feat"]).reshape(N_POD, T * F)
    sf = np.asarray(inputs["svc_feat"]).reshape(N_SVC, T * F)

    in_src = np.asarray(inputs["inst_node_src"]).astype(np.int64)
    in_dst = np.asarray(inputs["inst_node_dst"]).astype(np.int64)
    ni_src = np.asarray(inputs["node_inst_src"]).astype(np.int64)
    ni_dst = np.asarray(inputs["node_inst_dst"]).astype(np.int64)
    sc_src = np.asarray(inputs["svc_call_src"]).astype(np.int64)
    sc_dst = np.asarray(inputs["svc_call_dst"]).astype(np.int64)

    # normalization: x/sqrt(deg_out) -> segsum -> /sqrt(deg_in), folded per-edge
    ro_in, ri_in = _degrees(in_src, in_dst, N_POD, N_NODE)
    ro_ni, ri_ni = _degrees(ni_src, ni_dst, N_NODE, N_POD)
    ro_sc, ri_sc = _degrees(sc_src, sc_dst, N_SVC, N_SVC)

    # Route edges: per (core, tile) buckets.
    # tile order within core: pods tiles 0..29, svc 30..32, node 33 (node last)
    def route(src, dst, w, kind):
        if kind == 0:    # dst = node -> last tile (heaviest; keeps LSTM ramp fast)
            core = dst // NODE_PC
            q = dst - core * NODE_PC
            tile = np.full_like(dst, POD_TILES + SVC_TILES)
            row = q
        elif kind == 1:  # dst = pod -> tiles [0, POD_TILES)
            core = dst // POD_PC
            q = dst - core * POD_PC
            tile = q // P
            row = q % P
        else:            # dst = svc -> tiles [POD_TILES, POD_TILES+SVC_TILES)
            core = dst // SVC_PC
            q = dst - core * SVC_PC
            tile = POD_TILES + q // P
            row = q % P
        return core, tile, row, src, w

    ew_in = (ro_in[in_src] * ri_in[in_dst]).astype(np.float32)
    ew_ni = (ro_ni[ni_src] * ri_ni[ni_dst]).astype(np.float32)
    ew_sc = (ro_sc[sc_src] * ri_sc[sc_dst]).astype(np.float32)

    routed = {
        0: route(in_src, in_dst, ew_in, 0),   # node phase: src = pods
        1: route(ni_src, ni_dst, ew_ni, 1),   # pod phase:  src = nodes
        2: route(sc_src, sc_dst, ew_sc, 2),   # svc phase:  src = svcs
    }

    # per (core, tile) edge lists
    buckets = [[([], [], []) for _ in range(N_TILES)] for _ in range(NCORES)]
    for kind in (0, 1, 2):
        core, tile, row, src, w = routed[kind]
        order = np.lexsort((row, tile, core))
        core, tile, row, src, w = core[order], tile[order], row[order], src[order], w[order]
        # group
        key = core * N_TILES + tile
        uniq, starts = np.unique(key, return_index=True)
        starts = list(starts) + [len(key)]
        for ui, k in enumerate(uniq):
            c, t = int(k) // N_TILES, int(k) % N_TILES
            s, e = starts[ui], starts[ui + 1]
            buckets[c][t] = (src[s:e], row[s:e], w[s:e])

    # static chunk counts per tile (max over cores), >= 1
    K = []
    for t in range(N_TILES):
        mx = 1
        for c in range(NCORES):
            mx = max(mx, (len(buckets[c][t][0]) + P - 1) // P)
        K.append(mx)
    base = np.concatenate([[0], np.cumsum(K)]).astype(np.int64)
    C_total = int(base[-1])

    # Source features laid out in EDGE ORDER (row-duplicated): chunk c of the
    # conv reads srcdup[c*128:(c+1)*128] with a plain contiguous DMA -- no
    # indirect gather (saves the SWDGE descriptor-generation cost on gpsimd).
    srcfeat = {0: pf, 1: nf, 2: sf}
    kind_num = [1] * POD_TILES + [2] * SVC_TILES + [0] * NODE_TILES

    in_maps = []
    for c in range(NCORES):
        edst = np.zeros((C_total, P), dtype=np.float32)
        ew = np.zeros((C_total, P), dtype=np.float32)
        srcdup = np.zeros((C_total * P, T * F), dtype=BF16)

        for t in range(N_TILES):
            src, row, w = buckets[c][t]
            n = len(src)
            b0 = int(base[t]) * P
            if n:
                srcdup[b0:b0 + n] = srcfeat[kind_num[t]][src].astype(BF16)
            edst.reshape(-1)[b0:b0 + n] = row
            ew.reshape(-1)[b0:b0 + n] = w

        m = {
            "srcdup": srcdup,
            "edst": np.ascontiguousarray(edst.T),
            "ew": np.ascontiguousarray(ew.T),
        }
        in_maps.append(m)

    # ---- weights (identical on all cores) ----
    def conv_w(Wname):
        W = np.asarray(inputs[Wname])  # (T, F, H)
        wt = W.transpose(1, 0, 2).reshape(F, T * H)  # (64, 2048) F-major
        return np.vstack([wt, wt]).astype(BF16)       # (128, 2048) vertical dup

    def conv_b8(bname):
        # per-t DoubleRow rank-1 bias. Both DR slots are used for a
        # two-term compensated sum: hi = fp8(b), lo = fp8(b - hi); the
        # matmul adds them, cutting fp8 quantization error ~16x for free.
        b = np.asarray(inputs[bname]).reshape(T, H)
        hi = b.astype(FP8)
        lo = (b - hi.astype(np.float32)).astype(FP8)
        out = np.empty((T, 2, H), dtype=FP8)
        out[:, 0, :] = hi
        out[:, 1, :] = lo
        return out.reshape(1, T * 2 * H)

    def lstm_w(Wname):
        # rows [i,f,g,o] -> [i,f,o,g]; g block doubled so tanh(g) = 2*sigmoid(2g)-1
        # lets one Sigmoid cover all four gate chunks.
        W = np.asarray(inputs[Wname])  # (512, in_dim)
        Wp = np.concatenate([W[0:128], W[128:256], W[384:512], 2.0 * W[256:384]], axis=0)
        return np.ascontiguousarray(Wp.T).astype(BF16)  # (in_dim, 512), [i,f,o,2g]

    def lstm_b8(b1, b2):
        b = np.asarray(inputs[b1]) + np.asarray(inputs[b2])
        bp = np.concatenate([b[0:128], b[128:256], b[384:512], 2.0 * b[256:384]])
        bp = bp.reshape(4, H)
        hi = bp.astype(FP8)
        lo = (bp - hi.astype(np.float32)).astype(FP8)
        out = np.empty((4, 2, H), dtype=FP8)
        out[:, 0, :] = hi
        out[:, 1, :] = lo
        return out.reshape(1, 4 * 2 * H)

    shared = {
        "wt_node": conv_w("W_in"), "wt_pod": conv_w("W_ni"), "wt_svc": conv_w("W_svc"),
        "cb8_node": conv_b8("b_in"), "cb8_pod": conv_b8("b_ni"), "cb8_svc": conv_b8("b_svc"),
        "wih0": lstm_w("Wih0"), "whh0": lstm_w("Whh0"),
        "wih1": lstm_w("Wih1"), "whh1": lstm_w("Whh1"),
        "bias8_0": lstm_b8("bih0", "bhh0"), "bias8_1": lstm_b8("bih1", "bhh1"),
        "ones8": np.ones((1, 1024), dtype=FP8),
        "iota": np.broadcast_to(np.arange(P, dtype=np.float32), (P, P)).copy(),
    }
    for m in in_maps:
        m.update(shared)

    meta = (C_total, tuple(K))
    return meta, in_maps


# ----------------------------------------------------------------------------
# Device program
# ----------------------------------------------------------------------------

def _build(meta):
    import concourse.bass as bass
    import concourse.tile as tile
    import concourse.mybir as mybir

    C_total, K = meta
    f32 = mybir.dt.float32
    bf16 = mybir.dt.float16
    fp16 = mybir.dt.float16
    fp8 = mybir.dt.float8e4
    i32 = mybir.dt.int32
    AF = mybir.ActivationFunctionType
    ALU = mybir.AluOpType
    DR = mybir.MatmulPerfMode.DoubleRow

    import concourse.bacc as bacc
    nc = bacc.Bacc("TRN2", target_bir_lowering=False, debug=False, enable_asserts=False)

    srcdup_d = nc.dram_tensor("srcdup", [C_total * P, T * F], bf16, kind="ExternalInput")
    edst_d = nc.dram_tensor("edst", [P, C_total], f32, kind="ExternalInput")
    ew_d = nc.dram_tensor("ew", [P, C_total], f32, kind="ExternalInput")
    wt_d = {k: nc.dram_tensor(f"wt_{k}", [P, T * H], bf16, kind="ExternalInput")
            for k in ("node", "pod", "svc")}
    cb8_d = {k: nc.dram_tensor(f"cb8_{k}", [1, T * 2 * H], fp8, kind="ExternalInput")
             for k in ("node", "pod", "svc")}
    wih_d = [nc.dram_tensor(f"wih{l}", [H, 512], bf16, kind="ExternalInput") for l in range(2)]
    whh_d = [nc.dram_tensor(f"whh{l}", [H, 512], bf16, kind="ExternalInput") for l in range(2)]
    bias8_d = [nc.dram_tensor(f"bias8_{l}", [1, 4 * 2 * H], fp8, kind="ExternalInput") for l in range(2)]
    ones8_d = nc.dram_tensor("ones8", [1, 1024], fp8, kind="ExternalInput")
    iota_d = nc.dram_tensor("iota", [P, P], f32, kind="ExternalInput")
    out_d = nc.dram_tensor("out", [P, T * R_CORE], bf16, kind="ExternalOutput")

    tile_kind = (["pod"] * POD_TILES + ["svc"] * SVC_TILES + ["node"] * NODE_TILES)
    base = np.concatenate([[0], np.cumsum(K)]).astype(int)
    NODE_TILE_IDX = POD_TILES + SVC_TILES

    with tile.TileContext(nc) as tc:
        with tc.tile_pool(name="dram", bufs=NJ, space="DRAM") as dramp, \
             tc.tile_pool(name="const", bufs=1) as constp:
            # x0 spill split per LSTM batch tile so the LSTM can start on tile j
            # as soon as its 4 conv row-tiles are written (pipeline the phases)
            x0p = [dramp.tile([P, T * B], bf16, tag="x0p", name=f"x0p_{j}")
                   for j, (r0, B) in enumerate(LSTM_TILES)]

            # load constants
            edst_sb = constp.tile([P, C_total], f32)
            ew_sb = constp.tile([P, C_total], f32)
            iota_sb = constp.tile([P, P], f32)
            nc.sync.dma_start(edst_sb[:], edst_d.ap())
            nc.sync.dma_start(ew_sb[:], ew_d.ap())
            nc.sync.dma_start(iota_sb[:], iota_d.ap())
            wt_sb, cb8_sb = {}, {}
            for k in ("node", "pod", "svc"):
                wt_sb[k] = constp.tile([P, T * H], bf16, name=f"wt_{k}_sb")
                cb8_sb[k] = constp.tile([1, T * 2 * H], fp8, name=f"cb8_{k}_sb")
                nc.sync.dma_start(wt_sb[k][:], wt_d[k].ap())
                nc.sync.dma_start(cb8_sb[k][:], cb8_d[k].ap())
            wih_sb, whh_sb, bias8_sb = [], [], []
            for l in range(2):
                wih_sb.append(constp.tile([H, 512], bf16, name=f"wih{l}_sb"))
                whh_sb.append(constp.tile([H, 512], bf16, name=f"whh{l}_sb"))
                bias8_sb.append(constp.tile([1, 4 * 2 * H], fp8, name=f"bias8_{l}_sb"))
                nc.sync.dma_start(wih_sb[l][:], wih_d[l].ap())
                nc.sync.dma_start(whh_sb[l][:], whh_d[l].ap())
                nc.sync.dma_start(bias8_sb[l][:], bias8_d[l].ap())
            ones8_sb = constp.tile([1, 1024], fp8)
            nc.sync.dma_start(ones8_sb[:], ones8_d.ap())

            srcdup_ap = srcdup_d.ap()

            # Conv + LSTM share one scope (and one PSUM pool) so the two
            # phases pipeline: LSTM batch-tile j starts once its 4 conv
            # row-tiles have spilled.
            with tc.tile_pool(name="gat", bufs=6) as gatp, \
                 tc.tile_pool(name="ssb", bufs=6) as ssbp, \
                 tc.tile_pool(name="psum", bufs=2, space="PSUM") as psump, \
                 tc.tile_pool(name="aggsb", bufs=3) as aggsbp, \
                 tc.tile_pool(name="x0sb", bufs=3) as x0sbp, \
                 tc.tile_pool(name="x0res", bufs=3) as x0resp, \
                 tc.tile_pool(name="st_h0", bufs=NJ) as ph0, \
                 tc.tile_pool(name="st_c", bufs=2) as pcm, \
                 tc.tile_pool(name="st_h1", bufs=NJ) as ph1, \
                 tc.tile_pool(name="ifo", bufs=3) as ifop, \
                 tc.tile_pool(name="gt", bufs=4) as gtp, \
                 tc.tile_pool(name="og", bufs=11) as otp, \
                 tc.tile_pool(name="s2c", bufs=2) as s2cp, \
                 tc.tile_pool(name="tmp1", bufs=3) as t1p, \
                 tc.tile_pool(name="tmp2", bufs=3) as t2p, \
                 tc.tile_pool(name="xin", bufs=6) as xinp:

                def bias_mm(out_ap, lhsT_flat, n, start, stop):
                    # rank-1 bias add at half PE cost: fp8 DoubleRow with
                    # lhsT = [bias;0] pairs, rhs = ones
                    nc.tensor.matmul(
                        out=out_ap,
                        lhsT=lhsT_flat.rearrange("o (two m) -> o two m", two=2),
                        rhs=ones8_sb[:, 0:2 * n].rearrange("o (two m) -> o two m", two=2),
                        start=start, stop=stop, perf_mode=DR)

                def conv_group(d, k0, k1, gs, ss, agg, R):
                    # PE-accumulate chunk group [k0,k1] into one PSUM partial
                    # (m-outer so per-bank psum accumulation groups stay
                    # sequential), then DVE-combine into agg.
                    pp = psump.tile([P, T * H], f32, tag="ps", name=f"pp_{d}_{k0}")
                    for m in range(8):
                        for kk in range(k0, k1 + 1):
                            nc.tensor.matmul(
                                out=pp[:, m * R:(m + 1) * R],
                                lhsT=gs[kk][:, m * P:(m + 1) * P],
                                rhs=ss[kk][:], start=(kk == k0), stop=(kk == k1))
                    if k0 == 0:
                        nc.vector.tensor_copy(agg[:], pp[:, 0:8 * R])
                    else:
                        nc.vector.tensor_tensor(out=agg[:], in0=agg[:],
                                                in1=pp[:, 0:8 * R], op=ALU.add)
                    return pp

                N_RES = 3  # LSTM batch tiles whose x0 stays SBUF-resident
                x0res = [x0resp.tile([P, T * 512], bf16, tag="x0r", name=f"x0res_{j}")
                         for j in range(N_RES)]

                def conv_tail(d, kind, agg, R, hx):
                    # linear per timestep + bias (fp8 DoubleRow rank-1) into the
                    # tile's (dead) segsum psum tile -- one ring slot per conv
                    # tile instead of two -- then LeakyReLU -> fp16 -> spill
                    wt = wt_sb[kind]
                    for t in range(T):
                        pb = 64 * (t % 2)
                        bias_mm(hx[:, t * R:(t + 1) * R],
                                cb8_sb[kind][:, t * 2 * H:(t + 1) * 2 * H], R,
                                start=True, stop=False)
                        nc.tensor.matmul(
                            out=hx[:, t * R:(t + 1) * R],
                            lhsT=wt[pb:pb + F, t * H:(t + 1) * H],
                            rhs=agg[pb:pb + F, (t // 2) * R:(t // 2 + 1) * R],
                            start=False, stop=True)
                    j = min(d // 4, NJ - 1)
                    rl = P * (d - 4 * j)
                    if j < N_RES:
                        # Prelu writes straight into the resident tile's
                        # [h, t*512 + rl + r] slices; no DRAM round trip
                        dst = x0res[j][:].rearrange("h (t r) -> h t r", t=T)[:, :, rl:rl + R]
                        nc.scalar.activation(
                            dst, hx[:, 0:T * R].rearrange("h (t r) -> h t r", t=T),
                            AF.Prelu, alpha=0.01)
                        return
                    x0t = x0sbp.tile([P, T * R], bf16, tag="x0", name=f"x0t_{d}")
                    nc.scalar.activation(x0t[:], hx[:, 0:T * R], AF.Prelu, alpha=0.01)
                    # spill as one contiguous per-partition block (4KB runs, no
                    # sub-512B DMA penalty in the DMA-bound conv window); the
                    # strided cost moves to the LSTM-phase load where DMA is idle
                    nc.sync.dma_start(
                        x0p[j][:, rl * T:rl * T + R * T], x0t[:])

                gather_rr = [0]

                def conv_tile(d):
                    kind = tile_kind[d]
                    Kd = K[d]
                    R = P
                    agg = aggsbp.tile([P, 8 * R], bf16, tag="agg", name=f"agg_{d}")
                    gs, ss = [], []
                    for ki in range(Kd):
                        col = int(base[d]) + ki
                        g = gatp.tile([P, T * F], bf16, tag="g", name=f"g_{d}_{ki}")
                        # round-robin the gather issue over 2 DGE queues
                        eng = (nc.sync, nc.scalar)[gather_rr[0] % 2]
                        gather_rr[0] += 1
                        eng.dma_start(g[:], srcdup_ap[col * P:(col + 1) * P, :])
                        s = ssbp.tile([P, R], bf16, tag="s", name=f"s_{d}_{ki}")
                        nc.vector.tensor_scalar(
                            out=s[:], in0=iota_sb[:, 0:R],
                            scalar1=edst_sb[:, col:col + 1], scalar2=ew_sb[:, col:col + 1],
                            op0=ALU.is_equal, op1=ALU.mult)
                        gs.append(g)
                        ss.append(s)
                        if ki % 4 == 3 or ki == Kd - 1:
                            pp = conv_group(d, (ki // 4) * 4, ki, gs, ss, agg, R)
                    conv_tail(d, kind, agg, R, pp)

                # ---------------- LSTM phase ----------------
                h = [[None] * NJ, [None] * NJ]
                for j, (r0, B) in enumerate(LSTM_TILES):
                    h[0][j] = ph0.tile([P, B], bf16, tag="h0", name=f"h0_{j}")
                    h[1][j] = ph1.tile([P, B], bf16, tag="h1", name=f"h1_{j}")
                # merged cell-state per layer: [P, (j,B)] fp16, slices at j*512
                cm = [pcm.tile([P, 4608], fp16, tag="c", name=f"c_{l}") for l in range(2)]

                outr = out_d.ap().rearrange("h (t r) -> h t r", t=T)

                def cell_front(l, j, t, xin_l, os_, crit=False):
                    # gates [i,f,o,2g] via PE; one Sigmoid covers all four
                    # chunks (g weights pre-doubled; tanh(g) = 2*sigmoid(2g)-1).
                    # The freshest operand goes LAST in the matmul group: L0's
                    # hh (h from the sigma2c at this block's start), L1's ih
                    # (h0 of the same t) -- PE pre-runs the other matmuls.
                    B = LSTM_TILES[j][1]
                    gates = psump.tile([P, 4 * B], f32, tag="ps",
                                       name=f"gates_{l}_{j}_{t}")
                    for i in range(4):
                        bias_mm(gates[:, i * B:(i + 1) * B],
                                bias8_sb[l][:, i * 2 * H:(i + 1) * 2 * H], B,
                                start=True, stop=False)
                        mms = [(wih_sb[l], xin_l)]
                        if t > 0:
                            hh = (whh_sb[l], h[l][j][:])
                            mms = [hh, mms[0]] if l == 1 else [mms[0], hh]
                        for mi, (w, rhs) in enumerate(mms):
                            nc.tensor.matmul(
                                out=gates[:, i * B:(i + 1) * B],
                                lhsT=w[:, i * H:(i + 1) * H],
                                rhs=rhs, start=False, stop=(mi == len(mms) - 1))
                    sg = ifop.tile([P, 4 * B], bf16, tag="ifo", name=f"sg_{l}_{j}_{t}")
                    nc.scalar.activation(sg[:], gates[:], AF.Sigmoid)
                    # o-gate extracted so sg can recycle fast (small ring)
                    o = otp.tile([P, B], bf16, tag="o", name=f"o_{l}_{j}_{t}")
                    nc.vector.tensor_copy(o[:], sg[:, 2 * B:3 * B])
                    os_[j] = o
                    gt = gtp.tile([P, B], bf16, tag="gt", name=f"gt_{l}_{j}_{t}")
                    nc.vector.tensor_scalar(
                        out=gt[:], in0=sg[:, 3 * B:4 * B], scalar1=2.0, scalar2=-1.0,
                        op0=ALU.mult, op1=ALU.add)
                    cs = cm[l][:, j * 512:j * 512 + B]
                    if t == 0:
                        nc.vector.tensor_mul(cs, sg[:, 0:B], gt[:])
                    else:
                        t1 = t1p.tile([P, B], fp16, tag="t1", name=f"t1_{l}_{j}_{t}")
                        nc.vector.tensor_mul(t1[:], sg[:, B:2 * B], cs)
                        t2 = t2p.tile([P, B], bf16, tag="t2", name=f"t2_{l}_{j}_{t}")
                        nc.vector.tensor_mul(t2[:], sg[:, 0:B], gt[:])
                        if crit:
                            nc.vector.tensor_tensor(cs, t1[:], t2[:], op=ALU.add)
                        else:
                            nc.gpsimd.tensor_tensor(cs, t1[:], t2[:], op=ALU.add)

                def sigma2c_tail(l, t, quad, os_):
                    # merged tanh over the quad's c columns in ONE activation
                    # call, then h = sig(o)*tanh(c) per tile
                    q0 = min(quad) * 512
                    q1 = max(j * 512 + LSTM_TILES[j][1] for j in quad)
                    tcm = s2cp.tile([P, q1 - q0], bf16, tag="s2", name=f"tc_{l}_{t}_{quad[0]}")
                    nc.scalar.activation(tcm[:], cm[l][:, q0:q1], AF.Tanh)
                    for j in quad:
                        r0, B = LSTM_TILES[j]
                        nc.vector.tensor_mul(
                            h[l][j][:], os_[j][:], tcm[:, j * 512 - q0:j * 512 - q0 + B])
                        if l == 1:
                            nc.sync.dma_start(
                                outr[:, t:t + 1, r0:r0 + B].rearrange("h t r -> h (t r)"),
                                h[1][j][:])

                # per-t emission; q1/q2 sigma2c tails deferred into the next
                # layer's Act stream so Act never stalls on the c-update chain
                pending = [None]

                def lstm_t(t):
                    xs = {}
                    for j in range(NJ):
                        if j < N_RES:
                            xs[j] = x0res[j][:, t * 512:(t + 1) * 512]
                            continue
                        B = LSTM_TILES[j][1]
                        x = xinp.tile([P, B], bf16, tag="x", name=f"x_{t}_{j}")
                        nc.sync.dma_start(
                            x[:].rearrange("h (dl t r) -> h dl t r", t=1, r=P),
                            x0p[j][:].rearrange("h (dl t r) -> h dl t r", t=T, r=P)
                            [:, :, t:t + 1, :])
                        xs[j] = x
                    for l in range(2):
                        os_ = {}
                        for j in (0, 1):
                            cell_front(l, j, t, xs[j] if l == 0 else h[0][j][:], os_)
                        if pending[0] is not None:
                            pl, pt, pos = pending[0]
                            sigma2c_tail(pl, pt, QUADS[1], pos)
                            sigma2c_tail(pl, pt, QUADS[2], pos)
                            pending[0] = None
                        cell_front(l, 2, t, xs[2] if l == 0 else h[0][2][:], os_)
                        cell_front(l, 3, t, xs[3] if l == 0 else h[0][3][:], os_,
                                   crit=True)
                        cell_front(l, 8, t, xs[8] if l == 0 else h[0][8][:], os_,
                                   crit=True)
                        sigma2c_tail(l, t, QUADS[0], os_)
                        for j in (4, 5, 6, 7):
                            cell_front(l, j, t, xs[j] if l == 0 else h[0][j][:], os_,
                                       crit=(j >= 6))
                        pending[0] = (l, t, os_)

                for d in range(N_TILES):
                    conv_tile(d)
                for t in range(T):
                    lstm_t(t)
                pl, pt, pos = pending[0]
                sigma2c_tail(pl, pt, QUADS[1], pos)
                sigma2c_tail(pl, pt, QUADS[2], pos)

    nc.compile()
    return nc


# ----------------------------------------------------------------------------
# Entry points
# ----------------------------------------------------------------------------

def _assemble(results):
    # per-core out: (128, T*R_CORE) viewed [h, t*R_CORE + r] -> (r, t, h)
    full = np.empty((N_NODE + N_POD + N_SVC, T, H), dtype=np.float32)
    parts_node, parts_pod, parts_svc = [], [], []
    for cidx, res in enumerate(results):
        o = res["out"].astype(np.float32).reshape(H, T, R_CORE).transpose(2, 1, 0)  # (r, t, h)
        n_node = min(NODE_PC, max(0, N_NODE - cidx * NODE_PC))
        n_svc = min(SVC_PC, max(0, N_SVC - cidx * SVC_PC))
        parts_pod.append(o[0:POD_PC])
        svc0 = POD_TILES * P
        parts_svc.append(o[svc0:svc0 + n_svc])
        node0 = (POD_TILES + SVC_TILES) * P
        parts_node.append(o[node0:node0 + n_node])
    full[0:N_NODE] = np.concatenate(parts_node, axis=0)
    full[N_NODE:N_NODE + N_POD] = np.concatenate(parts_pod, axis=0)
    full[N_NODE + N_POD:] = np.concatenate(parts_svc, axis=0)
    return full


def run(inputs, trace=False):
    from concourse.bass_utils import run_bass_kernel_spmd
    meta, in_maps = _prep(inputs)
    if meta not in _COMPILED:
        _COMPILED[meta] = _build(meta)
    nc = _COMPILED[meta]
    try:
        res = run_bass_kernel_spmd(nc, in_maps, core_ids=list(range(NCORES)), trace=trace)
    except Exception:
        # transient device errors (e.g. NRT_EXEC_UNIT_UNRECOVERABLE) recover
        # on re-execution; retry once before giving up
        res = run_bass_kernel_spmd(nc, in_maps, core_ids=list(range(NCORES)), trace=trace)
    return _assemble(res.results), res


def kernel(**inputs):
    out, _ = run(inputs, trace=False)
    return out


# revision 55
# speedup vs baseline: 1.0115x; 1.0078x over previous
"""Trainium2 Bass kernel for nn_AggrHGraphConvWindow (3x GraphConv -> LeakyReLU -> 2-layer LSTM).

Contract: kernel(**inputs) takes FULL unsharded numpy inputs, returns FULL output
(33500, 16, 128) float32.  Internally shards destination rows across 8 NeuronCores
(graph/data parallel per the sharding hint: edges partitioned by destination with
halo exchange of source features), runs one SPMD Bass program, and gathers.
"""

import os
import numpy as np
import ml_dtypes

BF16 = np.float16  # fp16: same cost as bf16 on PE/DVE, 8x finer mantissa
FP8 = ml_dtypes.float8_e4m3

# Problem constants (hardcoded per spec)
N_NODE, N_POD, N_SVC = 500, 30000, 3000
T, F, H = 16, 64, 128
NCORES = 8
P = 128

NODE_PC = 64     # nodes per core (64*8=512 >= 500)
POD_PC = 3750    # pods per core (exact)
SVC_PC = 376     # svcs per core (376*8=3008 >= 3000)

POD_TILES = (POD_PC + P - 1) // P   # 30 (tile 29: 38 pods + 64 nodes packed)
SVC_TILES = (SVC_PC + P - 1) // P   # 3
N_TILES = POD_TILES + SVC_TILES     # 33
NODE_ROW0 = POD_PC - (POD_TILES - 1) * P  # 38: node rows start in tile 29
R_CORE = N_TILES * P  # 4224 rows per core (padded)

# LSTM batch tiles over the 4352 local rows
LSTM_TILES = [(j * 512, 512) for j in range(R_CORE // 512)]
if R_CORE % 512:
    LSTM_TILES.append((512 * (R_CORE // 512), R_CORE % 512))
NJ = len(LSTM_TILES)
QUADS = [(0, 1, 2, 3), (4, 5, 6, 7), (8,)]

_COMPILED = {}


# ----------------------------------------------------------------------------
# Host-side preprocessing: edge routing, degree norms, halo tables, weight prep
# ----------------------------------------------------------------------------

def _degrees(src, dst, n_src, n_dst):
    dout = np.bincount(src, minlength=n_src).astype(np.float64)
    din = np.bincount(dst, minlength=n_dst).astype(np.float64)
    return (1.0 / np.sqrt(np.maximum(dout, 1.0)), 1.0 / np.sqrt(np.maximum(din, 1.0)))


def _prep(inputs):
    nf = np.asarray(inputs["node_feat"]).reshape(N_NODE, T * F)
    pf = np.asarray(inputs["pod_feat"]).reshape(N_POD, T * F)
    sf = np.asarray(inputs["svc_feat"]).reshape(N_SVC, T * F)

    in_src = np.asarray(inputs["inst_node_src"]).astype(np.int64)
    in_dst = np.asarray(inputs["inst_node_dst"]).astype(np.int64)
    ni_src = np.asarray(inputs["node_inst_src"]).astype(np.int64)
    ni_dst = np.asarray(inputs["node_inst_dst"]).astype(np.int64)
    sc_src = np.asarray(inputs["svc_call_src"]).astype(np.int64)
    sc_dst = np.asarray(inputs["svc_call_dst"]).astype(np.int64)

    # normalization: x/sqrt(deg_out) -> segsum -> /sqrt(deg_in), folded per-edge
    ro_in, ri_in = _degrees(in_src, in_dst, N_POD, N_NODE)
    ro_ni, ri_ni = _degrees(ni_src, ni_dst, N_NODE, N_POD)
    ro_sc, ri_sc = _degrees(sc_src, sc_dst, N_SVC, N_SVC)

    # Route edges: per (core, tile) buckets.
    # tile order within core: pods tiles 0..29, svc 30..32, node 33 (node last)
    def route(src, dst, w, kind):
        if kind == 0:    # dst = node -> packed into pod tile 29's pad rows
            core = dst // NODE_PC
            q = dst - core * NODE_PC
            tile = np.full_like(dst, POD_TILES - 1)
            row = NODE_ROW0 + q
        elif kind == 1:  # dst = pod -> tiles [0, POD_TILES)
            core = dst // POD_PC
            q = dst - core * POD_PC
            tile = q // P
            row = q % P
        else:            # dst = svc -> tiles [POD_TILES, POD_TILES+SVC_TILES)
            core = dst // SVC_PC
            q = dst - core * SVC_PC
            tile = POD_TILES + q // P
            row = q % P
        return core, tile, row, src, w

    ew_in = (ro_in[in_src] * ri_in[in_dst]).astype(np.float32)
    ew_ni = (ro_ni[ni_src] * ri_ni[ni_dst]).astype(np.float32)
    ew_sc = (ro_sc[sc_src] * ri_sc[sc_dst]).astype(np.float32)

    routed = {
        0: route(in_src, in_dst, ew_in, 0),   # node phase: src = pods
        1: route(ni_src, ni_dst, ew_ni, 1),   # pod phase:  src = nodes
        2: route(sc_src, sc_dst, ew_sc, 2),   # svc phase:  src = svcs
    }

    # per (core, tile) edge lists (kind kept per edge: tile 29 mixes phases)
    allc, allt, allr, alls, allw, allk = [], [], [], [], [], []
    for kind in (0, 1, 2):
        core, tile, row, src, w = routed[kind]
        allc.append(core); allt.append(tile); allr.append(row)
        alls.append(src); allw.append(w)
        allk.append(np.full_like(src, kind))
    core = np.concatenate(allc); tile = np.concatenate(allt)
    row = np.concatenate(allr); src = np.concatenate(alls)
    w = np.concatenate(allw); kd = np.concatenate(allk)
    order = np.lexsort((row, tile, core))
    core, tile, row, src, w, kd = (a[order] for a in (core, tile, row, src, w, kd))
    buckets = [[(np.zeros(0, np.int64),) * 4 for _ in range(N_TILES)] for _ in range(NCORES)]
    key = core * N_TILES + tile
    uniq, starts = np.unique(key, return_index=True)
    starts = list(starts) + [len(key)]
    for ui, k in enumerate(uniq):
        c, t = int(k) // N_TILES, int(k) % N_TILES
        s, e = starts[ui], starts[ui + 1]
        buckets[c][t] = (src[s:e], row[s:e], w[s:e], kd[s:e])

    # static chunk counts per tile (max over cores), >= 1
    K = []
    for t in range(N_TILES):
        mx = 1
        for c in range(NCORES):
            mx = max(mx, (len(buckets[c][t][0]) + P - 1) // P)
        K.append(mx)
    base = np.concatenate([[0], np.cumsum(K)]).astype(np.int64)
    C_total = int(base[-1])

    # Source features laid out in EDGE ORDER (row-duplicated): chunk c of the
    # conv reads srcdup[c*128:(c+1)*128] with a plain contiguous DMA -- no
    # indirect gather (saves the SWDGE descriptor-generation cost on gpsimd).
    srcfeat = {0: pf, 1: nf, 2: sf}

    in_maps = []
    for c in range(NCORES):
        edst = np.zeros((C_total, P), dtype=np.float32)
        ew = np.zeros((C_total, P), dtype=np.float32)
        srcdup = np.zeros((C_total * P, T * F), dtype=BF16)

        for t in range(N_TILES):
            src, row, w, kde = buckets[c][t]
            n = len(src)
            b0 = int(base[t]) * P
            for kk in (0, 1, 2):
                mkk = kde == kk
                if mkk.any():
                    srcdup[b0:b0 + n][mkk] = srcfeat[kk][src[mkk]].astype(BF16)
            edst.reshape(-1)[b0:b0 + n] = row
            ew.reshape(-1)[b0:b0 + n] = w

        m = {
            "srcdup": srcdup,
            "edst": np.ascontiguousarray(edst.T),
            "ew": np.ascontiguousarray(ew.T),
        }
        in_maps.append(m)

    # ---- weights (identical on all cores) ----
    def conv_w(Wname):
        W = np.asarray(inputs[Wname])  # (T, F, H)
        wt = W.transpose(1, 0, 2).reshape(F, T * H)  # (64, 2048) F-major
        return np.vstack([wt, wt]).astype(BF16)       # (128, 2048) vertical dup

    def conv_b8(bname):
        # per-t DoubleRow rank-1 bias. Both DR slots are used for a
        # two-term compensated sum: hi = fp8(b), lo = fp8(b - hi); the
        # matmul adds them, cutting fp8 quantization error ~16x for free.
        b = np.asarray(inputs[bname]).reshape(T, H)
        hi = b.astype(FP8)
        lo = (b - hi.astype(np.float32)).astype(FP8)
        out = np.empty((T, 2, H), dtype=FP8)
        out[:, 0, :] = hi
        out[:, 1, :] = lo
        return out.reshape(1, T * 2 * H)

    def lstm_w(Wname):
        # rows [i,f,g,o] -> [i,f,o,g]; g block doubled so tanh(g) = 2*sigmoid(2g)-1
        # lets one Sigmoid cover all four gate chunks.
        W = np.asarray(inputs[Wname])  # (512, in_dim)
        Wp = np.concatenate([W[0:128], W[128:256], W[384:512], 2.0 * W[256:384]], axis=0)
        return np.ascontiguousarray(Wp.T).astype(BF16)  # (in_dim, 512), [i,f,o,2g]

    def lstm_b8(b1, b2):
        b = np.asarray(inputs[b1]) + np.asarray(inputs[b2])
        bp = np.concatenate([b[0:128], b[128:256], b[384:512], 2.0 * b[256:384]])
        bp = bp.reshape(4, H)
        hi = bp.astype(FP8)
        lo = (bp - hi.astype(np.float32)).astype(FP8)
        out = np.empty((4, 2, H), dtype=FP8)
        out[:, 0, :] = hi
        out[:, 1, :] = lo
        return out.reshape(1, 4 * 2 * H)

    shared = {
        "wt_node": conv_w("W_in"), "wt_pod": conv_w("W_ni"), "wt_svc": conv_w("W_svc"),
        "cb8_node": conv_b8("b_in"), "cb8_pod": conv_b8("b_ni"), "cb8_svc": conv_b8("b_svc"),
        "wih0": lstm_w("Wih0"), "whh0": lstm_w("Whh0"),
        "wih1": lstm_w("Wih1"), "whh1": lstm_w("Whh1"),
        "bias8_0": lstm_b8("bih0", "bhh0"), "bias8_1": lstm_b8("bih1", "bhh1"),
        "ones8": np.ones((1, 1024), dtype=FP8),
        "iota": np.broadcast_to(np.arange(P, dtype=np.float32), (P, P)).copy(),
    }
    for m in in_maps:
        m.update(shared)

    meta = (C_total, tuple(K))
    return meta, in_maps


# ----------------------------------------------------------------------------
# Device program
# ----------------------------------------------------------------------------

def _build(meta):
    import concourse.bass as bass
    import concourse.tile as tile
    import concourse.mybir as mybir

    C_total, K = meta
    f32 = mybir.dt.float32
    bf16 = mybir.dt.float16
    fp16 = mybir.dt.float16
    fp8 = mybir.dt.float8e4
    i32 = mybir.dt.int32
    AF = mybir.ActivationFunctionType
    ALU = mybir.AluOpType
    DR = mybir.MatmulPerfMode.DoubleRow

    import concourse.bacc as bacc
    nc = bacc.Bacc("TRN2", target_bir_lowering=False, debug=False, enable_asserts=False)

    srcdup_d = nc.dram_tensor("srcdup", [C_total * P, T * F], bf16, kind="ExternalInput")
    edst_d = nc.dram_tensor("edst", [P, C_total], f32, kind="ExternalInput")
    ew_d = nc.dram_tensor("ew", [P, C_total], f32, kind="ExternalInput")
    wt_d = {k: nc.dram_tensor(f"wt_{k}", [P, T * H], bf16, kind="ExternalInput")
            for k in ("node", "pod", "svc")}
    cb8_d = {k: nc.dram_tensor(f"cb8_{k}", [1, T * 2 * H], fp8, kind="ExternalInput")
             for k in ("node", "pod", "svc")}
    wih_d = [nc.dram_tensor(f"wih{l}", [H, 512], bf16, kind="ExternalInput") for l in range(2)]
    whh_d = [nc.dram_tensor(f"whh{l}", [H, 512], bf16, kind="ExternalInput") for l in range(2)]
    bias8_d = [nc.dram_tensor(f"bias8_{l}", [1, 4 * 2 * H], fp8, kind="ExternalInput") for l in range(2)]
    ones8_d = nc.dram_tensor("ones8", [1, 1024], fp8, kind="ExternalInput")
    iota_d = nc.dram_tensor("iota", [P, P], f32, kind="ExternalInput")
    out_d = nc.dram_tensor("out", [P, T * R_CORE], bf16, kind="ExternalOutput")

    # per-tile weight segments (kind, col0, col1); tile 29 mixes pod+node
    segments = [[("pod", 0, P)] for _ in range(POD_TILES - 1)]
    segments.append([("pod", 0, NODE_ROW0), ("node", NODE_ROW0, P)])
    segments += [[("svc", 0, P)] for _ in range(SVC_TILES)]
    base = np.concatenate([[0], np.cumsum(K)]).astype(int)

    with tile.TileContext(nc) as tc:
        with tc.tile_pool(name="dram", bufs=NJ, space="DRAM") as dramp, \
             tc.tile_pool(name="const", bufs=1) as constp:
            # x0 spill split per LSTM batch tile so the LSTM can start on tile j
            # as soon as its 4 conv row-tiles are written (pipeline the phases)
            x0p = [dramp.tile([P, T * B], bf16, tag="x0p", name=f"x0p_{j}")
                   for j, (r0, B) in enumerate(LSTM_TILES)]

            # load constants
            edst_sb = constp.tile([P, C_total], f32)
            ew_sb = constp.tile([P, C_total], f32)
            iota_sb = constp.tile([P, P], f32)
            nc.sync.dma_start(edst_sb[:], edst_d.ap())
            nc.sync.dma_start(ew_sb[:], ew_d.ap())
            nc.sync.dma_start(iota_sb[:], iota_d.ap())
            wt_sb, cb8_sb = {}, {}
            for k in ("node", "pod", "svc"):
                wt_sb[k] = constp.tile([P, T * H], bf16, name=f"wt_{k}_sb")
                cb8_sb[k] = constp.tile([1, T * 2 * H], fp8, name=f"cb8_{k}_sb")
                nc.sync.dma_start(wt_sb[k][:], wt_d[k].ap())
                nc.sync.dma_start(cb8_sb[k][:], cb8_d[k].ap())
            wih_sb, whh_sb, bias8_sb = [], [], []
            for l in range(2):
                wih_sb.append(constp.tile([H, 512], bf16, name=f"wih{l}_sb"))
                whh_sb.append(constp.tile([H, 512], bf16, name=f"whh{l}_sb"))
                bias8_sb.append(constp.tile([1, 4 * 2 * H], fp8, name=f"bias8_{l}_sb"))
                nc.sync.dma_start(wih_sb[l][:], wih_d[l].ap())
                nc.sync.dma_start(whh_sb[l][:], whh_d[l].ap())
                nc.sync.dma_start(bias8_sb[l][:], bias8_d[l].ap())
            ones8_sb = constp.tile([1, 1024], fp8)
            nc.sync.dma_start(ones8_sb[:], ones8_d.ap())

            srcdup_ap = srcdup_d.ap()

            # Conv + LSTM share one scope (and one PSUM pool) so the two
            # phases pipeline: LSTM batch-tile j starts once its 4 conv
            # row-tiles have spilled.
            with tc.tile_pool(name="gat", bufs=6) as gatp, \
                 tc.tile_pool(name="ssb", bufs=6) as ssbp, \
                 tc.tile_pool(name="psum", bufs=2, space="PSUM") as psump, \
                 tc.tile_pool(name="aggsb", bufs=3) as aggsbp, \
                 tc.tile_pool(name="x0sb", bufs=3) as x0sbp, \
                 tc.tile_pool(name="x0res", bufs=3) as x0resp, \
                 tc.tile_pool(name="st_h0", bufs=NJ) as ph0, \
                 tc.tile_pool(name="st_c", bufs=2) as pcm, \
                 tc.tile_pool(name="st_h1", bufs=NJ) as ph1, \
                 tc.tile_pool(name="ifo", bufs=3) as ifop, \
                 tc.tile_pool(name="gt", bufs=4) as gtp, \
                 tc.tile_pool(name="og", bufs=11) as otp, \
                 tc.tile_pool(name="s2c", bufs=2) as s2cp, \
                 tc.tile_pool(name="tmp1", bufs=3) as t1p, \
                 tc.tile_pool(name="tmp2", bufs=3) as t2p, \
                 tc.tile_pool(name="xin", bufs=6) as xinp:

                def bias_mm(out_ap, lhsT_flat, n, start, stop):
                    # rank-1 bias add at half PE cost: fp8 DoubleRow with
                    # lhsT = [bias;0] pairs, rhs = ones
                    nc.tensor.matmul(
                        out=out_ap,
                        lhsT=lhsT_flat.rearrange("o (two m) -> o two m", two=2),
                        rhs=ones8_sb[:, 0:2 * n].rearrange("o (two m) -> o two m", two=2),
                        start=start, stop=stop, perf_mode=DR)

                def conv_group(d, k0, k1, gs, ss, agg, R):
                    # PE-accumulate chunk group [k0,k1] into one PSUM partial
                    # (m-outer so per-bank psum accumulation groups stay
                    # sequential), then DVE-combine into agg.
                    pp = psump.tile([P, T * H], f32, tag="ps", name=f"pp_{d}_{k0}")
                    for m in range(8):
                        for kk in range(k0, k1 + 1):
                            nc.tensor.matmul(
                                out=pp[:, m * R:(m + 1) * R],
                                lhsT=gs[kk][:, m * P:(m + 1) * P],
                                rhs=ss[kk][:], start=(kk == k0), stop=(kk == k1))
                    if k0 == 0:
                        nc.vector.tensor_copy(agg[:], pp[:, 0:8 * R])
                    else:
                        nc.vector.tensor_tensor(out=agg[:], in0=agg[:],
                                                in1=pp[:, 0:8 * R], op=ALU.add)
                    return pp

                N_RES = 3  # LSTM batch tiles whose x0 stays SBUF-resident
                x0res = [x0resp.tile([P, T * 512], bf16, tag="x0r", name=f"x0res_{j}")
                         for j in range(N_RES)]

                def conv_tail(d, segs, agg, R, hx):
                    # linear per timestep + bias (fp8 DoubleRow rank-1) into the
                    # tile's (dead) segsum psum tile -- one ring slot per conv
                    # tile instead of two -- then LeakyReLU -> fp16 -> spill.
                    # segs: per-column-range weight kinds (tile 29 mixes
                    # pod+node rows)
                    for t in range(T):
                        pb = 64 * (t % 2)
                        for kind, c0, c1 in segs:
                            bias_mm(hx[:, t * R + c0:t * R + c1],
                                    cb8_sb[kind][:, t * 2 * H:(t + 1) * 2 * H],
                                    c1 - c0, start=True, stop=False)
                            nc.tensor.matmul(
                                out=hx[:, t * R + c0:t * R + c1],
                                lhsT=wt_sb[kind][pb:pb + F, t * H:(t + 1) * H],
                                rhs=agg[pb:pb + F, (t // 2) * R + c0:(t // 2) * R + c1],
                                start=False, stop=True)
                    j = min(d // 4, NJ - 1)
                    rl = P * (d - 4 * j)
                    if j < N_RES:
                        # Prelu writes straight into the resident tile's
                        # [h, t*512 + rl + r] slices; no DRAM round trip
                        dst = x0res[j][:].rearrange("h (t r) -> h t r", t=T)[:, :, rl:rl + R]
                        nc.scalar.activation(
                            dst, hx[:, 0:T * R].rearrange("h (t r) -> h t r", t=T),
                            AF.Prelu, alpha=0.01)
                        return
                    x0t = x0sbp.tile([P, T * R], bf16, tag="x0", name=f"x0t_{d}")
                    nc.scalar.activation(x0t[:], hx[:, 0:T * R], AF.Prelu, alpha=0.01)
                    # spill as one contiguous per-partition block (4KB runs, no
                    # sub-512B DMA penalty in the DMA-bound conv window); the
                    # strided cost moves to the LSTM-phase load where DMA is idle
                    nc.sync.dma_start(
                        x0p[j][:, rl * T:rl * T + R * T], x0t[:])

                gather_rr = [0]

                def conv_tile(d):
                    Kd = K[d]
                    R = P
                    agg = aggsbp.tile([P, 8 * R], bf16, tag="agg", name=f"agg_{d}")
                    gs, ss = [], []
                    for ki in range(Kd):
                        col = int(base[d]) + ki
                        g = gatp.tile([P, T * F], bf16, tag="g", name=f"g_{d}_{ki}")
                        # round-robin the gather issue over 2 DGE queues
                        eng = (nc.sync, nc.scalar)[gather_rr[0] % 2]
                        gather_rr[0] += 1
                        eng.dma_start(g[:], srcdup_ap[col * P:(col + 1) * P, :])
                        s = ssbp.tile([P, R], bf16, tag="s", name=f"s_{d}_{ki}")
                        nc.vector.tensor_scalar(
                            out=s[:], in0=iota_sb[:, 0:R],
                            scalar1=edst_sb[:, col:col + 1], scalar2=ew_sb[:, col:col + 1],
                            op0=ALU.is_equal, op1=ALU.mult)
                        gs.append(g)
                        ss.append(s)
                        if ki % 4 == 3 or ki == Kd - 1:
                            pp = conv_group(d, (ki // 4) * 4, ki, gs, ss, agg, R)
                    conv_tail(d, segments[d], agg, R, pp)

                # ---------------- LSTM phase ----------------
                h = [[None] * NJ, [None] * NJ]
                for j, (r0, B) in enumerate(LSTM_TILES):
                    h[0][j] = ph0.tile([P, B], bf16, tag="h0", name=f"h0_{j}")
                    h[1][j] = ph1.tile([P, B], bf16, tag="h1", name=f"h1_{j}")
                # merged cell-state per layer: [P, (j,B)] fp16, slices at j*512
                cm = [pcm.tile([P, 4608], fp16, tag="c", name=f"c_{l}") for l in range(2)]

                outr = out_d.ap().rearrange("h (t r) -> h t r", t=T)

                def cell_front(l, j, t, xin_l, os_, crit=False):
                    # gates [i,f,o,2g] via PE; one Sigmoid covers all four
                    # chunks (g weights pre-doubled; tanh(g) = 2*sigmoid(2g)-1).
                    # The freshest operand goes LAST in the matmul group: L0's
                    # hh (h from the sigma2c at this block's start), L1's ih
                    # (h0 of the same t) -- PE pre-runs the other matmuls.
                    B = LSTM_TILES[j][1]
                    gates = psump.tile([P, 4 * B], f32, tag="ps",
                                       name=f"gates_{l}_{j}_{t}")
                    for i in range(4):
                        bias_mm(gates[:, i * B:(i + 1) * B],
                                bias8_sb[l][:, i * 2 * H:(i + 1) * 2 * H], B,
                                start=True, stop=False)
                        mms = [(wih_sb[l], xin_l)]
                        if t > 0:
                            hh = (whh_sb[l], h[l][j][:])
                            mms = [hh, mms[0]] if l == 1 else [mms[0], hh]
                        for mi, (w, rhs) in enumerate(mms):
                            nc.tensor.matmul(
                                out=gates[:, i * B:(i + 1) * B],
                                lhsT=w[:, i * H:(i + 1) * H],
                                rhs=rhs, start=False, stop=(mi == len(mms) - 1))
                    sg = ifop.tile([P, 4 * B], bf16, tag="ifo", name=f"sg_{l}_{j}_{t}")
                    nc.scalar.activation(sg[:], gates[:], AF.Sigmoid)
                    # o-gate extracted so sg can recycle fast (small ring)
                    o = otp.tile([P, B], bf16, tag="o", name=f"o_{l}_{j}_{t}")
                    nc.vector.tensor_copy(o[:], sg[:, 2 * B:3 * B])
                    os_[j] = o
                    gt = gtp.tile([P, B], bf16, tag="gt", name=f"gt_{l}_{j}_{t}")
                    nc.vector.tensor_scalar(
                        out=gt[:], in0=sg[:, 3 * B:4 * B], scalar1=2.0, scalar2=-1.0,
                        op0=ALU.mult, op1=ALU.add)
                    cs = cm[l][:, j * 512:j * 512 + B]
                    if t == 0:
                        nc.vector.tensor_mul(cs, sg[:, 0:B], gt[:])
                    else:
                        t1 = t1p.tile([P, B], fp16, tag="t1", name=f"t1_{l}_{j}_{t}")
                        nc.vector.tensor_mul(t1[:], sg[:, B:2 * B], cs)
                        t2 = t2p.tile([P, B], bf16, tag="t2", name=f"t2_{l}_{j}_{t}")
                        nc.vector.tensor_mul(t2[:], sg[:, 0:B], gt[:])
                        if crit:
                            nc.vector.tensor_tensor(cs, t1[:], t2[:], op=ALU.add)
                        else:
                            nc.gpsimd.tensor_tensor(cs, t1[:], t2[:], op=ALU.add)

                def sigma2c_tail(l, t, quad, os_):
                    # merged tanh over the quad's c columns in ONE activation
                    # call, then h = sig(o)*tanh(c) per tile
                    q0 = min(quad) * 512
                    q1 = max(j * 512 + LSTM_TILES[j][1] for j in quad)
                    tcm = s2cp.tile([P, q1 - q0], bf16, tag="s2", name=f"tc_{l}_{t}_{quad[0]}")
                    nc.scalar.activation(tcm[:], cm[l][:, q0:q1], AF.Tanh)
                    for j in quad:
                        r0, B = LSTM_TILES[j]
                        nc.vector.tensor_mul(
                            h[l][j][:], os_[j][:], tcm[:, j * 512 - q0:j * 512 - q0 + B])
                        if l == 1:
                            nc.sync.dma_start(
                                outr[:, t:t + 1, r0:r0 + B].rearrange("h t r -> h (t r)"),
                                h[1][j][:])

                # per-t emission; q1/q2 sigma2c tails deferred into the next
                # layer's Act stream so Act never stalls on the c-update chain
                pending = [None]

                def lstm_t(t):
                    xs = {}
                    for j in range(NJ):
                        if j < N_RES:
                            xs[j] = x0res[j][:, t * 512:(t + 1) * 512]
                            continue
                        B = LSTM_TILES[j][1]
                        x = xinp.tile([P, B], bf16, tag="x", name=f"x_{t}_{j}")
                        nc.sync.dma_start(
                            x[:].rearrange("h (dl t r) -> h dl t r", t=1, r=P),
                            x0p[j][:].rearrange("h (dl t r) -> h dl t r", t=T, r=P)
                            [:, :, t:t + 1, :])
                        xs[j] = x
                    for l in range(2):
                        os_ = {}
                        for j in (0, 1):
                            cell_front(l, j, t, xs[j] if l == 0 else h[0][j][:], os_)
                        if pending[0] is not None:
                            pl, pt, pos = pending[0]
                            sigma2c_tail(pl, pt, QUADS[1], pos)
                            sigma2c_tail(pl, pt, QUADS[2], pos)
                            pending[0] = None
                        cell_front(l, 2, t, xs[2] if l == 0 else h[0][2][:], os_)
                        cell_front(l, 3, t, xs[3] if l == 0 else h[0][3][:], os_,
                                   crit=True)
                        cell_front(l, 8, t, xs[8] if l == 0 else h[0][8][:], os_,
                                   crit=True)
                        sigma2c_tail(l, t, QUADS[0], os_)
                        for j in (4, 5, 6, 7):
                            cell_front(l, j, t, xs[j] if l == 0 else h[0][j][:], os_,
                                       crit=(j >= 6))
                        pending[0] = (l, t, os_)

                for d in range(N_TILES):
                    conv_tile(d)
                for t in range(T):
                    lstm_t(t)
                pl, pt, pos = pending[0]
                sigma2c_tail(pl, pt, QUADS[1], pos)
                sigma2c_tail(pl, pt, QUADS[2], pos)

    nc.compile()
    return nc


# ----------------------------------------------------------------------------
# Entry points
# ----------------------------------------------------------------------------

def _assemble(results):
    # per-core out: (128, T*R_CORE) viewed [h, t*R_CORE + r] -> (r, t, h)
    full = np.empty((N_NODE + N_POD + N_SVC, T, H), dtype=np.float32)
    parts_node, parts_pod, parts_svc = [], [], []
    for cidx, res in enumerate(results):
        o = res["out"].astype(np.float32).reshape(H, T, R_CORE).transpose(2, 1, 0)  # (r, t, h)
        n_node = min(NODE_PC, max(0, N_NODE - cidx * NODE_PC))
        n_svc = min(SVC_PC, max(0, N_SVC - cidx * SVC_PC))
        parts_pod.append(o[0:POD_PC])
        svc0 = POD_TILES * P
        parts_svc.append(o[svc0:svc0 + n_svc])
        node0 = (POD_TILES - 1) * P + NODE_ROW0
        parts_node.append(o[node0:node0 + n_node])
    full[0:N_NODE] = np.concatenate(parts_node, axis=0)
    full[N_NODE:N_NODE + N_POD] = np.concatenate(parts_pod, axis=0)
    full[N_NODE + N_POD:] = np.concatenate(parts_svc, axis=0)
    return full


def run(inputs, trace=False):
    from concourse.bass_utils import run_bass_kernel_spmd
    meta, in_maps = _prep(inputs)
    if meta not in _COMPILED:
        _COMPILED[meta] = _build(meta)
    nc = _COMPILED[meta]
    try:
        res = run_bass_kernel_spmd(nc, in_maps, core_ids=list(range(NCORES)), trace=trace)
    except Exception:
        # transient device errors (e.g. NRT_EXEC_UNIT_UNRECOVERABLE) recover
        # on re-execution; retry once before giving up
        res = run_bass_kernel_spmd(nc, in_maps, core_ids=list(range(NCORES)), trace=trace)
    return _assemble(res.results), res


def kernel(**inputs):
    out, _ = run(inputs, trace=False)
    return out
